# revision 1
# baseline (speedup 1.0000x reference)
"""AttentionalJoin kernel for 8 Trainium2 NeuronCores.

Math: the reference builds full (M x M) self-attention over M = N+1 tokens
(CLS prepended) but returns only the CLS row of the projected output.  Only
the CLS query survives, so attention collapses to a softmax-weighted token
pooling:

    q       = Wq @ cls                       (per head h: q_h)
    score_t = scale * q_h . (Wk x_t)_h  =  x_t . R[:, h],   R = scale*Wk_h^T q_h
    p       = softmax over the M tokens (scores bounded ~[-6, 6]; no max-sub)
    pooled_h = sum_t p_t x_t                 (linearity: project AFTER pooling)
    out     = proj( concat_h Wv_h pooled_h ) + proj_b

The device streams x once (memory-bound part): scores = X @ R, exp, and the
weighted token-sum + partition Z per head; x is streamed in fp16 (rel err
~3e-4, PSUM accumulation stays fp32).  X^T for the scores matmul is built
on the PE as plain identity matmuls (fp16 fast-weight-load, keeps HAM warm)
with PSUM->SBUF fp16 copies split across DVE/ACT.  The tiny tail (head-mix
with Wv, proj, bias, cls-token contribution — ~10 MFLOP on 256 KB) runs on
host in fp32.

Sharding: data-parallel over the batch dim, 2 batches per core.
"""

import numpy as np

H = 8
C = 512
HD = C // H
B = 16
N = 2048
NCORES = 8
BPC = B // NCORES          # batches per core
TOK = BPC * N              # tokens per core (4096)
NCHUNK = TOK // 512        # 512-token compute chunks per core (8)
MAX_DRAIN_WAITS = 1        # this walrus rejects instructions w/ >1 sem wait

_cached = {}


def _patch_drain():
    """The container's walrus codegen rejects instructions carrying more
    than one sem wait ("Too many sync wait commands").  Split extra waits
    onto dedicated same-engine NOPs, which preserves semantics (engine
    queues are in-order)."""
    import concourse.tile as tile_mod
    from concourse import mybir
    from bass_rust import ScopedClock

    if getattr(tile_mod.TileContext, "_drain_patched", False):
        return

    orig_lower = tile_mod.TileContext._lower_ordered_insts

    def _lower_ordered_insts(self, ordered):
        nc = self.nc
        for bbname, insts in ordered.items():
            out = []
            for inst in insts:
                si = inst.sync_info
                if si is not None and si.on_wait and len(si.on_wait) > MAX_DRAIN_WAITS:
                    waits = list(si.on_wait)
                    extra, keep = waits[:-MAX_DRAIN_WAITS], waits[-MAX_DRAIN_WAITS:]
                    for w in extra:
                        nop = mybir.InstNoOp(
                            name=f"waitsplit-{nc.next_id()}",
                            engine=inst.engine,
                            ins=[],
                            outs=[],
                            bass_nofuse=True,
                            sync_info=mybir.SyncInfo(on_wait=[w], on_update=[]),
                            debug=inst.debug,
                        )
                        out.append(nop)
                    inst.sync_info = mybir.SyncInfo(
                        on_wait=keep, on_update=list(si.on_update)
                    )
                out.append(inst)
            ordered[bbname] = out
        return orig_lower(self, ordered)

    tile_mod.TileContext._lower_ordered_insts = _lower_ordered_insts

    def _drain_and_barrier(self, tick_clock, wait_clock):
        nc = self.nc
        probe = mybir.InstNoOp(
            name=f"drain-wait-probe-{nc.next_id()}",
            engine=mybir.EngineType.SP,
            ins=[],
            outs=[],
        )
        wait_clock.add_sem_waits(probe, ScopedClock({None: tick_clock.global_clock}))
        waits = list(probe.sync_info.on_wait) if probe.sync_info else []
        for i in range(0, len(waits), MAX_DRAIN_WAITS):
            chunk = waits[i : i + MAX_DRAIN_WAITS]
            nop = nc.sync.nop(nofuse=True, hint="drain_wait")
            nop.ins.sync_info = mybir.SyncInfo(on_wait=chunk, on_update=[])
        nc.sync.drain()

        nc.all_engine_barrier()
        popped = nc._tile_sem_poison_stack.pop()
        assert popped is self._sem_poison
        nc.clear_and_free_semaphores(list(self.sems.allocated().values()))
        nc.all_engine_barrier()

    tile_mod.TileContext._drain_and_barrier = _drain_and_barrier
    tile_mod.TileContext._drain_patched = True


def _build_module():
    import concourse.bass as bass
    import concourse.tile as tile
    from concourse import mybir
    from concourse.masks import make_identity

    _patch_drain()
    f16 = mybir.dt.float16
    f32 = mybir.dt.float32
    EXP = mybir.ActivationFunctionType.Exp

    nc = bass.Bass()
    x_in = nc.dram_tensor("x", [TOK, C], f16, kind="ExternalInput")
    r_in = nc.dram_tensor("r", [C, H], f16, kind="ExternalInput")
    s_out = nc.dram_tensor("s", [BPC, H, C], f32, kind="ExternalOutput")
    z_out = nc.dram_tensor("z", [BPC, H, N // 512], f32, kind="ExternalOutput")

    # natural-layout view: 8 chunks of 512 tokens, 4 blocks of 128 each
    x_src = x_in.rearrange("(a j p) f -> a p j f", a=8, j=4, p=128)
    r_src = r_in.rearrange("(q p) h -> p q h", p=128)

    with tile.TileContext(nc) as tc:
        with (
            tc.tile_pool(name="xpool", bufs=1) as xpool,
            tc.tile_pool(name="consts", bufs=1) as consts,
            tc.tile_pool(name="xtpool", bufs=1) as xtpool,
            tc.tile_pool(name="epool", bufs=1) as epool,
            tc.tile_pool(name="opool", bufs=2) as opool,
            tc.tile_pool(name="pt", bufs=2, space="PSUM") as pt_pool,
            tc.tile_pool(name="psc", bufs=2, space="PSUM") as psc_pool,
            tc.tile_pool(name="pet", bufs=2, space="PSUM") as pet_pool,
            tc.tile_pool(name="ps", bufs=1, space="PSUM") as ps_pool,
        ):
            ident = consts.tile([128, 128], f16)
            make_identity(nc, ident)
            r_sb = consts.tile([128, 4, H], f16)
            nc.sync.dma_start(out=r_sb, in_=r_src)

            # x natural layout (rhs of the pooling matmul); 512KB chunks so
            # the first chunk lands early and the PE starts sooner
            x_sb = []
            for A in range(8):
                t = xpool.tile([128, 4, C], f16, tag=f"x{A}", name=f"x{A}")
                nc.sync.dma_start(out=t, in_=x_src[A])
                x_sb.append(t)

            # x^T built on the PE as plain matmuls against the identity:
            # out = x_blockT @ I  (fp16 weights -> fast weight load), then a
            # PSUM -> SBUF fp16 copy on DVE/ACT
            xt = [
                xtpool.tile([128, TOK], f16, tag=f"xt{q}", name=f"xt{q}")
                for q in range(4)
            ]

            e_sb = [epool.tile([H, N], f16, tag=f"e{b}", name=f"e{b}") for b in range(BPC)]
            zp = [
                epool.tile([H, N // 512], f32, tag=f"zp{b}", name=f"zp{b}")
                for b in range(BPC)
            ]
            et = [
                epool.tile([128, 16, H], f16, tag=f"et{b}", name=f"et{b}")
                for b in range(BPC)
            ]
            ps = [ps_pool.tile([H, C], f32, tag=f"ps{b}", name=f"psacc{b}") for b in range(BPC)]

            ncopy = 0

            def stage1(a):
                """x^T for chunk a: 16 identity matmuls + 4 PSUM->SBUF copies."""
                nonlocal ncopy
                for q in range(4):
                    pt = pt_pool.tile([128, 512], f32, tag="pt", name=f"pt{a}_{q}")
                    for j in range(4):
                        nc.tensor.matmul(
                            pt[:, j * 128 : (j + 1) * 128],
                            x_sb[a][:, j, q * 128 : (q + 1) * 128],
                            ident,
                            start=True,
                            stop=True,
                        )
                    dst = xt[q][:, a * 512 : (a + 1) * 512]
                    if ncopy % 3 != 2:
                        nc.vector.tensor_copy(dst, pt)
                    else:
                        nc.scalar.copy(dst, pt)
                    ncopy += 1

            def stage2(a):
                nonlocal ncopy
                b, g = divmod(a, 4)
                psc = psc_pool.tile([H, 512], f32, tag="psc", name=f"psc{a}")
                for q in range(4):
                    nc.tensor.matmul(
                        psc,
                        r_sb[:, q, :],
                        xt[q][:, a * 512 : (a + 1) * 512],
                        start=(q == 0),
                        stop=(q == 3),
                    )
                nc.scalar.activation(
                    out=e_sb[b][:, g * 512 : (g + 1) * 512],
                    in_=psc,
                    func=EXP,
                    accum_out=zp[b][:, g : g + 1],
                )
                for jj in range(4):
                    j = g * 4 + jj
                    pet = pet_pool.tile([128, H], f32, tag="pet", name=f"pet{a}_{jj}")
                    # transpose E via a plain matmul: out = e_sliceT @ I8
                    nc.tensor.matmul(
                        pet,
                        e_sb[b][:, j * 128 : (j + 1) * 128],
                        ident[:H, :H],
                        start=True,
                        stop=True,
                    )
                    if ncopy % 2 == 0:
                        nc.vector.tensor_copy(et[b][:, j, :], pet)
                    else:
                        nc.scalar.copy(et[b][:, j, :], pet)
                    ncopy += 1
                for jj in range(4):
                    j = g * 4 + jj
                    nc.tensor.matmul(
                        ps[b],
                        et[b][:, j, :],
                        x_sb[a][:, jj, :],
                        start=(j == 0),
                        stop=(j == 15),
                    )

            def emit_out(b):
                so = opool.tile([H, C], f32, tag=f"so{b}", name=f"so{b}")
                nc.vector.tensor_copy(so, ps[b])
                nc.gpsimd.dma_start(out=s_out[b], in_=so)
                nc.gpsimd.dma_start(out=z_out[b], in_=zp[b])

            # software pipeline: transpose chunk a while chunk a-1 computes;
            # batch 0's result retires as soon as its accumulation closes
            stage1(0)
            for a in range(1, NCHUNK):
                stage1(a)
                stage2(a - 1)
                if a - 1 == 3:
                    emit_out(0)
            stage2(NCHUNK - 1)
            emit_out(1)

    return nc


def _get_module():
    if "nc" not in _cached:
        _cached["nc"] = _build_module()
    return _cached["nc"]


def _host_prep(cls, qkv_w):
    scale = HD ** -0.5
    c = cls.reshape(C).astype(np.float64)
    Wq = qkv_w[:C].astype(np.float64)
    Wk = qkv_w[C : 2 * C].astype(np.float64)
    q = Wq @ c
    qh = q.reshape(H, HD)
    Wkh = Wk.reshape(H, HD, C)
    R = (scale * np.einsum("hdc,hd->ch", Wkh, qh)).astype(np.float16)
    k0 = Wk @ c
    score0 = scale * np.einsum("hd,hd->h", qh, k0.reshape(H, HD))
    e0 = np.exp(score0)
    return R, e0


def kernel(x, cls, qkv_w, proj_w, proj_b):
    from concourse.bass_utils import run_bass_kernel_spmd

    x = np.asarray(x, dtype=np.float32)
    cls = np.asarray(cls, dtype=np.float32)
    qkv_w = np.asarray(qkv_w, dtype=np.float32)
    proj_w = np.asarray(proj_w, dtype=np.float32)
    proj_b = np.asarray(proj_b, dtype=np.float32)

    R, e0 = _host_prep(cls, qkv_w)
    Wv = qkv_w[2 * C :]

    x16 = np.ascontiguousarray(x.reshape(B * N, C).astype(np.float16))
    nc = _get_module()
    in_maps = [
        {"x": x16[i * TOK : (i + 1) * TOK], "r": R}
        for i in range(NCORES)
    ]
    res = run_bass_kernel_spmd(nc, in_maps, list(range(NCORES)))
    _cached["last_results"] = res

    s_dev = np.concatenate([res.results[i]["s"] for i in range(NCORES)], axis=0)
    z_dev = np.concatenate(
        [res.results[i]["z"].sum(axis=-1) for i in range(NCORES)], axis=0
    )

    # add the CLS token's own contribution, normalize, head-mix + proj
    cf = cls.reshape(C)
    s_full = s_dev + (e0[:, None] * cf[None, :]).astype(np.float32)[None]
    z_full = z_dev + e0.astype(np.float32)[None]
    v = s_full / z_full[:, :, None]
    o = np.einsum("hdc,bhc->bhd", Wv.reshape(H, HD, C), v).reshape(B, C)
    y = o @ proj_w.T + proj_b
    return y.astype(np.float32)



# revision 3
# speedup vs baseline: 1.8361x; 1.8361x over previous
"""AttentionalJoin kernel for 8 Trainium2 NeuronCores.

Math: the reference builds full (M x M) self-attention over M = N+1 tokens
(CLS prepended) but returns only the CLS row of the projected output.  Only
the CLS query survives, so attention collapses to a softmax-weighted token
pooling:

    q       = Wq @ cls                       (per head h: q_h)
    score_t = scale * q_h . (Wk x_t)_h  =  x_t . R[:, h],   R = scale*Wk_h^T q_h
    p       = softmax over the M tokens
    pooled_h = sum_t p_t x_t                 (linearity: project AFTER pooling)
    out     = proj( concat_h Wv_h pooled_h ) + proj_b

Device design (per core, 2 batches of 2048 tokens):
  * x is uploaded TWICE in fp8-e3m4 (scaled by 2 to dodge the e3m4 subnormal
    cliff): once c-major (xt, the scores operand) and once t-major with an
    appended ones column (xn, the pooling operand + partition function Z).
    Dual-layout upload removes every on-chip transpose: the baseline spent
    ~60% of its time building X^T on the PE and copying PSUM->SBUF.
  * scores: per 128-token block, xt blocks are the PE's STATIONARY operand
    (fast-weight-load, 128-col fp8) against a tiny bf16 rhs R[128c, 8h],
    accumulating E-layout [128t, 8h] directly in PSUM.  Mixed-dtype matmul
    (fp8 weights x bf16 moving) is supported by the PE and keeps R at bf16
    precision for free.
  * exp: one ACT activation per batch over [128, 16*8] PSUM -> bf16 E.
  * pooling: E blocks [128t, 8h] are the stationary operand (bf16) against
    the streamed xn [128t, 260] halves (fp8), accumulating pooled^T [8, 260]
    over the 16 blocks of each batch.  Column 256 is the ones column -> Z.
  * tail (CLS contribution, normalize, Wv head-mix, proj, bias: ~10 MFLOP)
    runs on host in fp64.

Sharding: data-parallel over the batch dim, 2 batches per core.
"""

import numpy as np

H = 8
C = 512
HD = C // H
B = 16
N = 2048
NCORES = 8
BPC = B // NCORES          # batches per core
TOK = BPC * N              # tokens per core (4096)
NBLK = TOK // 128          # 128-token blocks per core (32)
NBB = N // 128             # blocks per batch (16)
HALF = 260                 # pooling rhs half width: 256 x + 1 ones + 3 pad
XSC = 2.0                  # x is stored as 2*x in e3m4
MAX_DRAIN_WAITS = 1        # this walrus rejects instructions w/ >1 sem wait

_cached = {}


def _patch_drain():
    """The container's walrus codegen rejects instructions carrying more
    than one sem wait ("Too many sync wait commands").  Split extra waits
    onto dedicated same-engine NOPs, which preserves semantics (engine
    queues are in-order)."""
    import concourse.tile as tile_mod
    from concourse import mybir
    from bass_rust import ScopedClock

    if getattr(tile_mod.TileContext, "_drain_patched", False):
        return

    orig_lower = tile_mod.TileContext._lower_ordered_insts

    def _lower_ordered_insts(self, ordered):
        nc = self.nc
        for bbname, insts in ordered.items():
            out = []
            for inst in insts:
                si = inst.sync_info
                if si is not None and si.on_wait and len(si.on_wait) > MAX_DRAIN_WAITS:
                    waits = list(si.on_wait)
                    extra, keep = waits[:-MAX_DRAIN_WAITS], waits[-MAX_DRAIN_WAITS:]
                    for w in extra:
                        nop = mybir.InstNoOp(
                            name=f"waitsplit-{nc.next_id()}",
                            engine=inst.engine,
                            ins=[],
                            outs=[],
                            bass_nofuse=True,
                            sync_info=mybir.SyncInfo(on_wait=[w], on_update=[]),
                            debug=inst.debug,
                        )
                        out.append(nop)
                    inst.sync_info = mybir.SyncInfo(
                        on_wait=keep, on_update=list(si.on_update)
                    )
                out.append(inst)
            ordered[bbname] = out
        return orig_lower(self, ordered)

    tile_mod.TileContext._lower_ordered_insts = _lower_ordered_insts

    def _drain_and_barrier(self, tick_clock, wait_clock):
        nc = self.nc
        probe = mybir.InstNoOp(
            name=f"drain-wait-probe-{nc.next_id()}",
            engine=mybir.EngineType.SP,
            ins=[],
            outs=[],
        )
        wait_clock.add_sem_waits(probe, ScopedClock({None: tick_clock.global_clock}))
        waits = list(probe.sync_info.on_wait) if probe.sync_info else []
        for i in range(0, len(waits), MAX_DRAIN_WAITS):
            chunk = waits[i : i + MAX_DRAIN_WAITS]
            nop = nc.sync.nop(nofuse=True, hint="drain_wait")
            nop.ins.sync_info = mybir.SyncInfo(on_wait=chunk, on_update=[])
        nc.sync.drain()

        nc.all_engine_barrier()
        popped = nc._tile_sem_poison_stack.pop()
        assert popped is self._sem_poison
        nc.clear_and_free_semaphores(list(self.sems.allocated().values()))
        nc.all_engine_barrier()

    tile_mod.TileContext._drain_and_barrier = _drain_and_barrier
    tile_mod.TileContext._drain_patched = True


def _build_module():
    import concourse.bass as bass
    import concourse.tile as tile
    from concourse import mybir

    _patch_drain()
    f8e3 = mybir.dt.float8e3
    bf16 = mybir.dt.bfloat16
    f32 = mybir.dt.float32
    EXP = mybir.ActivationFunctionType.Exp

    nc = bass.Bass()
    # register a -1.0 constant for the exp bias (same recipe as Bass.__init__)
    _bias_t = nc.alloc_sbuf_tensor("const-f32-neg1", [128, 1], f32)
    nc.gpsimd.memset(_bias_t.ap(), -1.0)
    nc.const_aps.aps[(f32, -1.0)] = _bias_t.ap()
    nc.all_engine_barrier()

    xt_in = nc.dram_tensor("xt", [128, NBLK, 4, 128], f8e3, kind="ExternalInput")
    xn_in = nc.dram_tensor("xn", [128, BPC, NBB, 2, HALF], f8e3, kind="ExternalInput")
    r_in = nc.dram_tensor("r", [128, 4, H], bf16, kind="ExternalInput")
    pool_out = nc.dram_tensor("pool", [H, BPC * 2, HALF], f32, kind="ExternalOutput")

    with tile.TileContext(nc) as tc:
        with (
            tc.tile_pool(name="xtp", bufs=1) as xtp,
            tc.tile_pool(name="xnp", bufs=1) as xnp,
            tc.tile_pool(name="consts", bufs=1) as consts,
            tc.tile_pool(name="ep", bufs=1) as ep,
            tc.tile_pool(name="op", bufs=1) as op,
            tc.tile_pool(name="pss", bufs=1, space="PSUM") as pss,
            tc.tile_pool(name="psp", bufs=1, space="PSUM") as psp,
        ):
            r_sb = consts.tile([128, 4, H], bf16)
            nc.sync.dma_start(out=r_sb, in_=r_in[:, :, :])

            xt_sb = xtp.tile([128, NBLK, 4, 128], f8e3, name="xt_sb")
            xn_sb = xnp.tile([128, BPC, NBB, 2, HALF], f8e3, name="xn_sb")
            # chunked input DMA: xt in 4 chunks (8 blocks each), then xn in
            # 4 chunks (half-batch each) so scores start as early as possible
            for ch in range(4):
                nc.sync.dma_start(
                    out=xt_sb[:, ch * 8 : (ch + 1) * 8],
                    in_=xt_in[:, ch * 8 : (ch + 1) * 8],
                )
            for b in range(BPC):
                for hb in range(2):
                    nc.sync.dma_start(
                        out=xn_sb[:, b, hb * 8 : (hb + 1) * 8],
                        in_=xn_in[:, b, hb * 8 : (hb + 1) * 8],
                    )

            ps_s = [
                pss.tile([128, NBB * H], f32, name=f"ps_s{b}") for b in range(BPC)
            ]
            ps_p = [
                [psp.tile([H, HALF], f32, name=f"ps_p{b}_{h2}") for h2 in range(2)]
                for b in range(BPC)
            ]
            e_sb = [ep.tile([128, NBB, H], bf16, name=f"e{b}") for b in range(BPC)]
            out_sb = op.tile([H, BPC * 2, HALF], f32, name="out_sb")

            # ---- scores: E[t, h] = exp(x_t . R - 1), accumulated per block
            for b in range(BPC):
                for blk in range(NBB):
                    a = b * NBB + blk
                    dst = ps_s[b][:, blk * H : (blk + 1) * H]
                    for q in range(4):
                        nc.tensor.matmul(
                            dst,
                            xt_sb[:, a, q, :],
                            r_sb[:, q, :],
                            start=(q == 0),
                            stop=(q == 3),
                        )
                # one activation per batch: PSUM fp32 -> bf16 E
                nc.scalar.activation(
                    out=e_sb[b],
                    in_=ps_s[b].rearrange("p (i h) -> p i h", h=H),
                    func=EXP,
                    bias=-1.0,
                )

            # ---- pooling: pooled^T[h, c] = sum_t E[t, h] * xn[t, c]
            for b in range(BPC):
                for h2 in range(2):
                    for blk in range(NBB):
                        nc.tensor.matmul(
                            ps_p[b][h2],
                            e_sb[b][:, blk, :],
                            xn_sb[:, b, blk, h2, :],
                            start=(blk == 0),
                            stop=(blk == NBB - 1),
                        )
                    nc.vector.tensor_copy(out_sb[:, b * 2 + h2, :], ps_p[b][h2])
            nc.gpsimd.dma_start(out=pool_out[:, :, :], in_=out_sb)

    return nc


def _get_module():
    if "nc" not in _cached:
        _cached["nc"] = _build_module()
    return _cached["nc"]


def _host_prep(cls, qkv_w):
    """R (scores projection) and the CLS token's own score, in fp64."""
    scale = HD ** -0.5
    c = cls.reshape(C).astype(np.float64)
    Wq = qkv_w[:C].astype(np.float64)
    Wk = qkv_w[C : 2 * C].astype(np.float64)
    q = (Wq @ c).reshape(H, HD)
    Wkh = Wk.reshape(H, HD, C)
    R = scale * np.einsum("hdc,hd->ch", Wkh, q)           # (C, H)
    k0 = Wk @ c
    s0 = scale * np.einsum("hd,hd->h", q, k0.reshape(H, HD))
    return R, s0


def _shard_images(x):
    """Build the two per-core fp8 images of x (see module docstring)."""
    import ml_dtypes

    f8 = ml_dtypes.float8_e3m4
    x8 = np.ascontiguousarray(x.reshape(B * N, C) * np.float32(XSC)).astype(f8)
    xts, xns = [], []
    for i in range(NCORES):
        xc = x8[i * TOK : (i + 1) * TOK]                   # (4096, 512)
        # xt[p, a, q, j] = 2x[a*128+j, q*128+p]
        xt = np.ascontiguousarray(
            xc.reshape(NBLK, 128, 4, 128).transpose(3, 0, 2, 1)
        )
        # xn[p, b, i, h2, j<256] = 2x[b*2048+i*128+p, h2*256+j]; j=256 -> 1
        xn = np.zeros((128, BPC, NBB, 2, HALF), dtype=f8)
        xn[..., :256] = xc.reshape(BPC, NBB, 128, 2, 256).transpose(2, 0, 1, 3, 4)
        xn[..., 256] = f8(1.0)
        xts.append(xt)
        xns.append(xn)
    return xts, xns


def _in_maps(inputs):
    import ml_dtypes

    x = np.asarray(inputs["x"], dtype=np.float32)
    cls = np.asarray(inputs["cls"], dtype=np.float32)
    qkv_w = np.asarray(inputs["qkv_w"], dtype=np.float32)
    R, _ = _host_prep(cls, qkv_w)
    r_img = np.ascontiguousarray(
        (R / XSC).reshape(4, 128, H).transpose(1, 0, 2)
    ).astype(ml_dtypes.bfloat16)
    xts, xns = _shard_images(x)
    return [{"xt": xts[i], "xn": xns[i], "r": r_img} for i in range(NCORES)]


def kernel(x, cls, qkv_w, proj_w, proj_b):
    from concourse.bass_utils import run_bass_kernel_spmd

    x = np.asarray(x, dtype=np.float32)
    cls = np.asarray(cls, dtype=np.float32)
    qkv_w = np.asarray(qkv_w, dtype=np.float32)
    proj_w = np.asarray(proj_w, dtype=np.float32)
    proj_b = np.asarray(proj_b, dtype=np.float32)

    nc = _get_module()
    res = run_bass_kernel_spmd(nc, _in_maps({"x": x, "cls": cls, "qkv_w": qkv_w}),
                               list(range(NCORES)))
    _cached["last_results"] = res

    R, s0 = _host_prep(cls, qkv_w)
    e0 = np.exp(s0 - 1.0)                                  # matches device bias
    Wv = qkv_w[2 * C :].astype(np.float64)
    cf = cls.reshape(C).astype(np.float64)

    # pool[h, b*2+h2, j]: j<256 -> sum_t e_t * 2x[t, h2*256+j]; j=256 -> Z
    num = np.empty((B, H, C), dtype=np.float64)
    z = np.empty((B, H), dtype=np.float64)
    for i in range(NCORES):
        p = res.results[i]["pool"].astype(np.float64)      # (H, 4, HALF)
        for b in range(BPC):
            num[i * BPC + b, :, :256] = p[:, b * 2, :256] / XSC
            num[i * BPC + b, :, 256:] = p[:, b * 2 + 1, :256] / XSC
            z[i * BPC + b] = p[:, b * 2, 256]
    num += (e0[:, None] * cf[None, :])[None]
    z += e0[None]
    v = num / z[:, :, None]
    o = np.einsum("hdc,bhc->bhd", Wv.reshape(H, HD, C), v).reshape(B, C)
    y = o @ proj_w.T.astype(np.float64) + proj_b.astype(np.float64)
    return y.astype(np.float32)
